# revision 4
# baseline (speedup 1.0000x reference)
"""Trainium2 Bass kernel for BlockMLP.

Math (per block n of 64): out_n = gelu(x_n @ W1_n + b1_n) @ W2_n + b2_n
  x: [8192, 4096] viewed as 64 blocks of [8192, 64]
  W1: [64, 64, 256], W2: [64, 256, 64], biases broadcast over batch.

Sharding: block-parallel across the 8 cores (8 blocks = 4 block-pairs per
core, full 8192-row batch).  vs. batch-parallel this cuts per-core weight
DMA 8x (8 MiB -> 1 MiB), which removes the HBM saturation that starved the
scalar engine (the GELU bottleneck) during the first half of the run.
Each core's x slice is shipped feature-major (transposed on the host as
part of the shard staging), so the kernel needs no PE transposes and no
PSUM staging for them; that frees all 8 PSUM banks for the L1 outputs,
letting one GELU activation cover a whole block-pair (N=2048) instead of
two N=1024 instructions — the scalar engine is the critical path, so its
per-instruction overhead is what sets the kernel time.

Per core (feature slice of 512 = 4 pairs, 16 batch chunks of 512 rows):
  - xT tiles [128 feat, 512 batch] loaded with SWDGE cast fp32->bf16.
  - L1: W1 as stationary (row-packed: the pair's two K=64 stationaries in
    row groups 0-63/64-127), xT as moving (bf16).  All four matmuls of a
    pair (2 row groups x 2 hidden halves) land in one PSUM tile
    [128, 4, 512] f32 (4 banks; 2 such tiles = all 8 banks).
  - One GELU activation per pair (N=2048) straight out of PSUM, writing
    bf16 g^T to SBUF.  Biases are zero in the graded problem; nonzero
    biases fall back to the CPU path.
  - L2: g^T slices as stationary (the PE transposes the stationary, so the
    output comes out batch-major), W2 (bf16) as moving, accumulating the
    two K=128 halves in PSUM.  The L2 output reuses the first bank of the
    pair's L1 PSUM tile (write-after-read on the GELU input; the Tile
    framework tracks the hazard).
  - Single out DMA per (chunk, pair) via HWDGE.
"""

import numpy as np

BS = 8192
D = 4096
NB = 64   # total blocks
BD = 64   # block input/output dim
H = 256   # hidden dim per block
N_CORES = 8
NBC = NB // N_CORES   # 8 blocks per core
NP = NBC // 2         # 4 block pairs per core
DC = NBC * BD         # 512 feature columns per core
B = BS                # full batch on every core
BC = 512              # batch chunk (rows per inner iteration)
NT = BC // 128        # batch tiles of 128 within a chunk
CHUNKS = B // BC      # 16

_CACHE = {}


def _patch_tile_drain():
    """walrus in this toolchain rejects instructions carrying >2 sync waits;
    Tile's tail drain carries one wait per live logical processor.  Spread
    the waits across several SP drains (engine-serial order keeps the
    barrier semantics)."""
    import bass_rust as _bass_rust
    import concourse.tile as tile

    VectorClock = _bass_rust.VectorClock
    ScopedClock = _bass_rust.ScopedClock

    def _drain_and_barrier(self, tick_clock, wait_clock):
        gc = list(tick_clock.global_clock)
        nprocs = len(gc)
        for p in range(nprocs):
            if gc[p] == 0:
                continue
            partial = [0] * nprocs
            partial[p] = gc[p]
            d = self.nc.sync.drain()
            wait_clock.add_sem_waits(d.ins, ScopedClock({None: VectorClock(partial)}))
        self.nc.all_engine_barrier()
        assert self.sems is not None
        popped = self.nc._tile_sem_poison_stack.pop()
        assert popped is self._sem_poison
        self.nc.clear_and_free_semaphores(list(self.sems.allocated().values()))
        self.nc.all_engine_barrier()

    tile.TileContext._drain_and_barrier = _drain_and_barrier


def _split_sync_waits(nc, maxw=1):
    """walrus (CoreV3GenImpl setupSyncWait) rejects instructions with more
    than 2 sync waits.  Move excess waits onto preceding same-engine NoOps;
    engine program order preserves the semantics."""
    from concourse import mybir

    uid = 0
    for fn in nc.m.functions:
        for blk in fn.blocks:
            insts = blk.instructions
            out = []
            changed = False
            for inst in insts:
                si = inst.sync_info
                waits = list(si.on_wait) if si and si.on_wait else []
                lim = maxw
                if len(waits) > lim:
                    changed = True
                    excess, keep = waits[:-lim], waits[-lim:]
                    for j in range(0, len(excess), maxw):
                        nop = mybir.InstNoOp(
                            name=f"wsplit-{uid}", ins=[], outs=[]
                        )
                        uid += 1
                        nop.engine = inst.engine
                        nop.sync_info = mybir.SyncInfo(
                            on_wait=excess[j : j + maxw], on_update=[]
                        )
                        out.append(nop)
                    si.on_wait = keep
                out.append(inst)
            if changed:
                blk.instructions = out


def _build(reps=1):
    from contextlib import ExitStack

    import concourse.bass as bass
    import concourse.tile as tile
    from concourse import mybir

    _patch_tile_drain()

    f32 = mybir.dt.float32
    bf16 = mybir.dt.bfloat16
    GELU = mybir.ActivationFunctionType.Gelu

    nc = bass.Bass()
    # x arrives feature-major: [DC, B] (host transposes each core's slice)
    x = nc.dram_tensor("x", [DC, B], f32, kind="ExternalInput")
    W1 = nc.dram_tensor("W1", [NBC, BD, H], f32, kind="ExternalInput")
    W2 = nc.dram_tensor("W2", [NBC, H, BD], f32, kind="ExternalInput")
    out = nc.dram_tensor("out", [B, DC], f32, kind="ExternalOutput")

    with ExitStack() as ctx:
        tc = ctx.enter_context(tile.TileContext(nc))
        wpool = ctx.enter_context(tc.tile_pool(name="w", bufs=1))
        xtp = ctx.enter_context(tc.tile_pool(name="xt", bufs=6))
        gp = ctx.enter_context(tc.tile_pool(name="g", bufs=6))
        outp = ctx.enter_context(tc.tile_pool(name="osb", bufs=6))
        psp = ctx.enter_context(tc.tile_pool(name="ps", bufs=2, space="PSUM"))

        # W1 stationaries: [128, NP, H]; partitions 0-63 = even block of each
        # pair, 64-127 = odd block.  W2 moving operands: [128, NBC, 2, BD].
        w1sb = wpool.tile([128, NP, H], bf16, tag="w1")
        w1v = W1.rearrange("(p two) i o -> (two i) p o", two=2)
        w2sb = wpool.tile([128, NBC, 2, BD], bf16, tag="w2")
        w2v = W2.rearrange("n (h k) o -> k n h o", h=2)

        def stage1(c, p, xt=None):
            # load xT columns, L1 matmuls, GELU over the whole pair
            if xt is None:
                xt = xtp.tile([128, BC], bf16, tag="xt")
                nc.gpsimd.dma_start(
                    xt[:], x[128 * p : 128 * (p + 1), BC * c : BC * (c + 1)]
                )
            ps1 = psp.tile([128, 4, BC], f32, tag="ps1")
            for h in range(2):
                hs = slice(128 * h, 128 * (h + 1))
                nc.tensor.matmul(
                    ps1[:, 2 * h, :],
                    lhsT=w1sb[0:64, p, hs],
                    rhs=xt[0:64, :],
                    start=True,
                    stop=True,
                )
                nc.tensor.matmul(
                    ps1[:, 2 * h + 1, :],
                    lhsT=w1sb[64:128, p, hs],
                    rhs=xt[64:128, :],
                    start=True,
                    stop=True,
                )
            gt = gp.tile([128, 4, BC], bf16, tag="g")
            nc.scalar.activation(gt[:], ps1[:], GELU)
            # g[blk, h] view: tile index 2*h + blk
            g = {(blk, h): gt[:, 2 * h + blk, :] for blk in range(2) for h in range(2)}
            return g, ps1

        def stage2(c, p, g, ps1):
            # L2 matmuls (accumulation pairs back-to-back) + out copy.
            # Output PSUM reuses the first bank of this pair's L1 tile
            # (free after the GELU read).
            ps_out = ps1[:, 0, :]
            for t in range(NT):
                ts_ = slice(128 * t, 128 * (t + 1))
                for blk, n in ((0, 2 * p), (1, 2 * p + 1)):
                    dst = ps_out[:, 128 * t + 64 * blk : 128 * t + 64 * blk + 64]
                    nc.tensor.matmul(
                        dst,
                        lhsT=g[blk, 0][:, ts_],
                        rhs=w2sb[:, n, 0, :],
                        start=True,
                        stop=False,
                    )
                    nc.tensor.matmul(
                        dst,
                        lhsT=g[blk, 1][:, ts_],
                        rhs=w2sb[:, n, 1, :],
                        start=False,
                        stop=True,
                    )
            fs = slice(128 * p, 128 * (p + 1))
            src_ap = ps_out.rearrange("q (t f) -> q t f", f=128)
            osb = outp.tile([128, NT, 128], f32, tag="osb")
            nc.vector.tensor_copy(osb[:], src_ap)
            nc.sync.dma_start(ov[c][:, :, fs], osb[:])

        # batch-tiled view of out DRAM: [chunk, row-in-tile(128), tile, feat]
        ov = out.rearrange("(c t q) d -> c q t d", t=NT, q=128)

        iters = [(c, p) for c in range(CHUNKS) for p in range(NP)]
        prev = None
        for rep in range(reps):
            for i, (c, p) in enumerate(iters):
                xt = None
                if rep == 0 and i == 0:
                    # first x load leads the Pool emission queue so its DMA
                    # completion latency hides the weight emissions
                    xt = xtp.tile([128, BC], bf16, tag="xt", name="xt0")
                    nc.gpsimd.dma_start(xt[:], x[0:128, 0:BC])
                    nc.gpsimd.dma_start(w1sb[:], w1v)
                if rep == 0 and i == 1:
                    nc.gpsimd.dma_start(w2sb[:], w2v)
                g, ps1 = stage1(c, p, xt)
                if prev is not None:
                    stage2(*prev)
                prev = (c, p, g, ps1)
        stage2(*prev)

    _split_sync_waits(nc)
    return nc


class _Runner:
    """Compiled SPMD executor over the 8 NeuronCores (mirrors
    bass2jax.run_bass_via_pjrt's multi-core path, without output donation so
    the same staged buffers can be executed repeatedly for timing)."""

    def __init__(self, nc):
        import jax
        import numpy as np
        from jax.sharding import Mesh, PartitionSpec
        from jax.experimental.shard_map import shard_map

        from concourse import bass2jax, mybir

        bass2jax.install_neuronx_cc_hook()

        partition_name = (
            nc.partition_id_tensor.name if nc.partition_id_tensor else None
        )
        in_names, out_names, out_avals = [], [], []
        for alloc in nc.m.functions[0].allocations:
            if not isinstance(alloc, mybir.MemoryLocationSet):
                continue
            name = alloc.memorylocations[0].name
            if alloc.kind == "ExternalInput":
                if name != partition_name:
                    in_names.append(name)
            elif alloc.kind == "ExternalOutput":
                out_names.append(name)
                out_avals.append(
                    jax.core.ShapedArray(
                        tuple(alloc.tensor_shape), mybir.dt.np(alloc.dtype)
                    )
                )
        all_names = list(in_names) + list(out_names)
        if partition_name is not None:
            all_names.append(partition_name)

        def _body(*args):
            operands = list(args)
            if partition_name is not None:
                operands.append(bass2jax.partition_id_tensor())
            outs = bass2jax._bass_exec_p.bind(
                *operands,
                out_avals=tuple(out_avals),
                in_names=tuple(all_names),
                out_names=tuple(out_names),
                lowering_input_output_aliases=(),
                sim_require_finite=True,
                sim_require_nnan=True,
                nc=nc,
            )
            return tuple(outs)

        devices = jax.devices()[:N_CORES]
        if len(devices) < N_CORES:
            raise RuntimeError(
                f"need {N_CORES} NeuronCores, found {len(devices)} jax devices"
            )
        self.mesh = Mesh(np.asarray(devices), ("core",))
        nin = len(in_names) + len(out_names)
        self.fn = jax.jit(
            shard_map(
                _body,
                mesh=self.mesh,
                in_specs=(PartitionSpec("core"),) * nin,
                out_specs=(PartitionSpec("core"),) * len(out_names),
                check_rep=False,
            ),
            keep_unused=True,
        )
        self.in_names = in_names
        self.out_names = out_names
        self.out_avals = out_avals
        self.jax = jax

    def stage(self, in_maps):
        """Concatenate per-core inputs and put them on the device mesh."""
        import numpy as np
        from jax.sharding import NamedSharding, PartitionSpec

        sh = NamedSharding(self.mesh, PartitionSpec("core"))
        args = []
        for name in self.in_names:
            c = np.concatenate([m[name] for m in in_maps], axis=0)
            args.append(self.jax.device_put(c, sh))
        for av in self.out_avals:
            z = np.zeros((N_CORES * av.shape[0], *av.shape[1:]), av.dtype)
            args.append(self.jax.device_put(z, sh))
        return args

    def run(self, args):
        outs = self.fn(*args)
        self.jax.block_until_ready(outs)
        return outs

    def time(self, args, iters=8):
        import time

        self.run(args)  # warm
        t0 = time.perf_counter()
        outs = None
        for _ in range(iters):
            outs = self.fn(*args)
        self.jax.block_until_ready(outs)
        t_pipe = (time.perf_counter() - t0) / iters
        per_call = []
        for _ in range(iters):
            t0 = time.perf_counter()
            self.jax.block_until_ready(self.fn(*args))
            per_call.append(time.perf_counter() - t0)
        return t_pipe, min(per_call)


def _get_runner():
    if "runner" not in _CACHE:
        _CACHE["runner"] = _Runner(_build())
    return _CACHE["runner"]


def _in_maps(x, W1, b1, W2, b2):
    x = np.asarray(x, dtype=np.float32)
    W1 = np.asarray(W1, dtype=np.float32)
    W2 = np.asarray(W2, dtype=np.float32)
    maps = []
    for i in range(N_CORES):
        maps.append(
            {
                # ship the core's feature slice transposed (feature-major)
                "x": np.ascontiguousarray(x[:, i * DC : (i + 1) * DC].T),
                "W1": np.ascontiguousarray(W1[i * NBC : (i + 1) * NBC]),
                "W2": np.ascontiguousarray(W2[i * NBC : (i + 1) * NBC]),
            }
        )
    return maps


def _kernel_cpu(x, W1, b1, W2, b2):
    """Reference math on the jax CPU backend (safety fallback)."""
    import jax
    import jax.numpy as jnp

    with jax.default_device(jax.devices("cpu")[0]):
        h = jnp.asarray(x).reshape(BS, NB, BD).transpose(1, 0, 2)
        h = jnp.einsum("nbi,nio->nbo", h, jnp.asarray(W1)) + jnp.asarray(b1)
        h = jax.nn.gelu(h, approximate=False)
        h = jnp.einsum("nbi,nio->nbo", h, jnp.asarray(W2)) + jnp.asarray(b2)
        return np.asarray(h.transpose(1, 0, 2).reshape(BS, D), dtype=np.float32)


def kernel(x, W1, b1, W2, b2):
    try:
        if np.any(b1) or np.any(b2):
            return _kernel_cpu(x, W1, b1, W2, b2)
        r = _get_runner()
        args = r.stage(_in_maps(x, W1, b1, W2, b2))
        outs = r.run(args)
        full = np.asarray(outs[r.out_names.index("out")])
        # [8*8192, 512] -> [8192, 8, 512] -> [8192, 4096]
        full = full.reshape(N_CORES, BS, DC).transpose(1, 0, 2).reshape(BS, D)
        return np.ascontiguousarray(full)
    except Exception:
        import traceback

        traceback.print_exc()
        return _kernel_cpu(x, W1, b1, W2, b2)


# revision 9
# speedup vs baseline: 1.0780x; 1.0780x over previous
"""Trainium2 Bass kernel for BlockMLP.

Math (per block n of 64): out_n = gelu(x_n @ W1_n + b1_n) @ W2_n + b2_n
  x: [8192, 4096] viewed as 64 blocks of [8192, 64]
  W1: [64, 64, 256], W2: [64, 256, 64], biases broadcast over batch.

Sharding: block-parallel across the 8 cores (8 blocks = 4 block-pairs per
core, full 8192-row batch).  vs. batch-parallel this cuts per-core weight
DMA 8x (8 MiB -> 1 MiB), which removes the HBM saturation that starved the
scalar engine (the GELU bottleneck, ~133 us/core) during the first half of
the run.

Per core (feature slice of 512 = 4 pairs, 16 batch chunks of 512 rows):
  - PE-transpose x tiles into feature-major layout [feat, batch] (the matmul
    contraction runs over the partition dim), bf16 cast on the SWDGE load.
  - L1: W1 as stationary (row-packed: the pair's two K=64 stationaries in
    row groups 0-63/64-127), x^T as moving (bf16).  Output lands
    feature-major in PSUM f32.
  - GELU on the scalar engine straight out of PSUM (one activation per
    (pair, h-half), N=1024), writing bf16 g^T to SBUF.  Biases are zero in
    the graded problem; nonzero biases fall back to the CPU path.
  - L2: g^T slices as stationary (the PE transposes the stationary, so the
    output comes out batch-major), W2 (bf16) as moving, accumulating the two
    K=128 halves in PSUM.  No output transpose needed.
  - Single out DMA per (chunk, pair) via HWDGE.
"""

import numpy as np

BS = 8192
D = 4096
NB = 64   # total blocks
BD = 64   # block input/output dim
H = 256   # hidden dim per block
N_CORES = 8
NBC = NB // N_CORES   # 8 blocks per core
NP = NBC // 2         # 4 block pairs per core
DC = NBC * BD         # 512 feature columns per core
B = BS                # full batch on every core
BC = 512              # batch chunk (rows per inner iteration)
NT = BC // 128        # batch tiles of 128 within a chunk
CHUNKS = B // BC      # 16

_CACHE = {}


def _patch_tile_drain():
    """walrus in this toolchain rejects instructions carrying >2 sync waits;
    Tile's tail drain carries one wait per live logical processor.  Spread
    the waits across several SP drains (engine-serial order keeps the
    barrier semantics)."""
    import bass_rust as _bass_rust
    import concourse.tile as tile

    VectorClock = _bass_rust.VectorClock
    ScopedClock = _bass_rust.ScopedClock

    def _drain_and_barrier(self, tick_clock, wait_clock):
        gc = list(tick_clock.global_clock)
        nprocs = len(gc)
        for p in range(nprocs):
            if gc[p] == 0:
                continue
            partial = [0] * nprocs
            partial[p] = gc[p]
            d = self.nc.sync.drain()
            wait_clock.add_sem_waits(d.ins, ScopedClock({None: VectorClock(partial)}))
        self.nc.all_engine_barrier()
        assert self.sems is not None
        popped = self.nc._tile_sem_poison_stack.pop()
        assert popped is self._sem_poison
        self.nc.clear_and_free_semaphores(list(self.sems.allocated().values()))
        self.nc.all_engine_barrier()

    tile.TileContext._drain_and_barrier = _drain_and_barrier


def _split_sync_waits(nc, maxw=1):
    """walrus (CoreV3GenImpl setupSyncWait) rejects instructions with more
    than 2 sync waits.  Move excess waits onto preceding same-engine NoOps;
    engine program order preserves the semantics."""
    from concourse import mybir

    uid = 0
    for fn in nc.m.functions:
        for blk in fn.blocks:
            insts = blk.instructions
            out = []
            changed = False
            for inst in insts:
                si = inst.sync_info
                waits = list(si.on_wait) if si and si.on_wait else []
                lim = maxw
                if len(waits) > lim:
                    changed = True
                    excess, keep = waits[:-lim], waits[-lim:]
                    for j in range(0, len(excess), maxw):
                        nop = mybir.InstNoOp(
                            name=f"wsplit-{uid}", ins=[], outs=[]
                        )
                        uid += 1
                        nop.engine = inst.engine
                        nop.sync_info = mybir.SyncInfo(
                            on_wait=excess[j : j + maxw], on_update=[]
                        )
                        out.append(nop)
                    si.on_wait = keep
                out.append(inst)
            if changed:
                blk.instructions = out


def _build(reps=1, zero_bias=True):
    assert zero_bias, "device kernel only supports zero biases"
    from contextlib import ExitStack

    import concourse.bass as bass
    import concourse.tile as tile
    from concourse import mybir
    from concourse.masks import make_identity

    _patch_tile_drain()

    f32 = mybir.dt.float32
    bf16 = mybir.dt.bfloat16
    GELU = mybir.ActivationFunctionType.Gelu

    nc = bass.Bass()
    x = nc.dram_tensor("x", [B, DC], f32, kind="ExternalInput")
    W1 = nc.dram_tensor("W1", [NBC, BD, H], f32, kind="ExternalInput")
    W2 = nc.dram_tensor("W2", [NBC, H, BD], f32, kind="ExternalInput")
    out = nc.dram_tensor("out", [B, DC], f32, kind="ExternalOutput")

    with ExitStack() as ctx:
        tc = ctx.enter_context(tile.TileContext(nc))
        const = ctx.enter_context(tc.tile_pool(name="const", bufs=1))
        wpool = ctx.enter_context(tc.tile_pool(name="w", bufs=1))
        xnatp = ctx.enter_context(tc.tile_pool(name="xnat", bufs=4))
        xtp = ctx.enter_context(tc.tile_pool(name="xt", bufs=4))
        gp = ctx.enter_context(tc.tile_pool(name="g", bufs=8))
        outp = ctx.enter_context(tc.tile_pool(name="osb", bufs=6))
        ps_t = ctx.enter_context(tc.tile_pool(name="ps_t", bufs=2, space="PSUM"))
        ps_l1 = ctx.enter_context(tc.tile_pool(name="ps_l1", bufs=2, space="PSUM"))
        ps_l2 = ctx.enter_context(tc.tile_pool(name="ps_l2", bufs=2, space="PSUM"))

        # ---- constants / weights (loaded once) ----
        identb = const.tile([128, 128], bf16, tag="identb")
        make_identity(nc, identb[:])
        # W1 stationaries: [128, NP, H]; partitions 0-63 = even block of each
        # pair, 64-127 = odd block.  W2 moving operands: [128, NBC, 2, BD].
        w1sb = wpool.tile([128, NP, H], bf16, tag="w1")
        w1v = W1.rearrange("(p two) i o -> two i p o", two=2)
        w2sb = wpool.tile([128, NBC, 2, BD], bf16, tag="w2")
        w2v = W2.rearrange("n (h k) o -> k n h o", h=2)

        def load_w1():
            nc.gpsimd.dma_start(w1sb[0:64, :], w1v[0])
            nc.gpsimd.dma_start(w1sb[64:128, :], w1v[1])

        def load_w2():
            nc.gpsimd.dma_start(w2sb[:], w2v)

        # batch-tiled views of x / out DRAM:
        #   [chunk, row-in-tile(128), tile(NT), pair, feature]
        xv = x.rearrange("(c t q) (p f) -> c q t p f", t=NT, q=128, f=128)
        ov = out.rearrange("(c t q) d -> c q t d", t=NT, q=128)

        def stage1(c, p, xnat=None):
            # load x columns, transpose, L1 matmuls, GELU
            if xnat is None:
                xnat = xnatp.tile([128, NT, 128], bf16, tag="xnat")
                nc.gpsimd.dma_start(xnat[:], xv[c, :, :, p, :])
            ps_xt = ps_t.tile([128, BC], bf16, tag="ps_xt")
            for t in range(NT):
                nc.tensor.transpose(
                    ps_xt[:, 128 * t : 128 * (t + 1)], xnat[:, t, :], identb[:]
                )
            xt = xtp.tile([128, BC], bf16, tag="xt")
            nc.vector.tensor_copy(xt[:], ps_xt[:])

            g = {}
            for h in range(2):
                hs = slice(128 * h, 128 * (h + 1))
                ps1 = ps_l1.tile([128, 2, BC], f32, tag="ps1")
                nc.tensor.matmul(
                    ps1[:, 0, :],
                    lhsT=w1sb[0:64, p, hs],
                    rhs=xt[0:64, :],
                    start=True,
                    stop=True,
                )
                nc.tensor.matmul(
                    ps1[:, 1, :],
                    lhsT=w1sb[64:128, p, hs],
                    rhs=xt[64:128, :],
                    start=True,
                    stop=True,
                )
                gt = gp.tile([128, 2, BC], bf16, tag="g", name=f"g_{h}")
                nc.scalar.activation(gt[:], ps1[:], GELU)
                g[0, h] = gt[:, 0, :]
                g[1, h] = gt[:, 1, :]
            return g

        def stage2(c, p, g):
            # L2 matmuls (accumulation pairs back-to-back) + out copy
            na, nb_ = 2 * p, 2 * p + 1
            ps_out = ps_l2.tile([128, BC], f32, tag="ps_out")
            for t in range(NT):
                ts_ = slice(128 * t, 128 * (t + 1))
                for blk, n in ((0, na), (1, nb_)):
                    dst = ps_out[:, 128 * t + 64 * blk : 128 * t + 64 * blk + 64]
                    nc.tensor.matmul(
                        dst,
                        lhsT=g[blk, 0][:, ts_],
                        rhs=w2sb[:, n, 0, :],
                        start=True,
                        stop=False,
                    )
                    nc.tensor.matmul(
                        dst,
                        lhsT=g[blk, 1][:, ts_],
                        rhs=w2sb[:, n, 1, :],
                        start=False,
                        stop=True,
                    )
            fs = slice(128 * p, 128 * (p + 1))
            src_ap = ps_out[:].rearrange("q (t f) -> q t f", f=128)
            osb = outp.tile([128, NT, 128], f32, tag="osb")
            nc.vector.tensor_copy(osb[:], src_ap)
            nc.sync.dma_start(ov[c][:, :, fs], osb[:])

        iters = [(c, p) for c in range(CHUNKS) for p in range(NP)]
        prev = None
        for rep in range(reps):
            for i, (c, p) in enumerate(iters):
                xnat = None
                if rep == 0 and i == 0:
                    # first x load leads the Pool emission queue so its DMA
                    # completion latency hides the weight emissions
                    xnat = xnatp.tile([128, NT, 128], bf16, tag="xnat", name="xn0")
                    nc.gpsimd.dma_start(xnat[:], xv[c, :, :, p, :])
                    load_w1()
                if rep == 0 and i == 1:
                    load_w2()
                g = stage1(c, p, xnat)
                if prev is not None:
                    stage2(*prev)
                prev = (c, p, g)
        stage2(*prev)

    _split_sync_waits(nc)
    return nc


class _Runner:
    """Compiled SPMD executor over the 8 NeuronCores (mirrors
    bass2jax.run_bass_via_pjrt's multi-core path, without output donation so
    the same staged buffers can be executed repeatedly for timing)."""

    def __init__(self, nc):
        import jax
        import numpy as np
        from jax.sharding import Mesh, PartitionSpec
        from jax.experimental.shard_map import shard_map

        from concourse import bass2jax, mybir

        bass2jax.install_neuronx_cc_hook()

        partition_name = (
            nc.partition_id_tensor.name if nc.partition_id_tensor else None
        )
        in_names, out_names, out_avals = [], [], []
        for alloc in nc.m.functions[0].allocations:
            if not isinstance(alloc, mybir.MemoryLocationSet):
                continue
            name = alloc.memorylocations[0].name
            if alloc.kind == "ExternalInput":
                if name != partition_name:
                    in_names.append(name)
            elif alloc.kind == "ExternalOutput":
                out_names.append(name)
                out_avals.append(
                    jax.core.ShapedArray(
                        tuple(alloc.tensor_shape), mybir.dt.np(alloc.dtype)
                    )
                )
        all_names = list(in_names) + list(out_names)
        if partition_name is not None:
            all_names.append(partition_name)

        def _body(*args):
            operands = list(args)
            if partition_name is not None:
                operands.append(bass2jax.partition_id_tensor())
            outs = bass2jax._bass_exec_p.bind(
                *operands,
                out_avals=tuple(out_avals),
                in_names=tuple(all_names),
                out_names=tuple(out_names),
                lowering_input_output_aliases=(),
                sim_require_finite=True,
                sim_require_nnan=True,
                nc=nc,
            )
            return tuple(outs)

        devices = jax.devices()[:N_CORES]
        if len(devices) < N_CORES:
            raise RuntimeError(
                f"need {N_CORES} NeuronCores, found {len(devices)} jax devices"
            )
        self.mesh = Mesh(np.asarray(devices), ("core",))
        nin = len(in_names) + len(out_names)
        self.fn = jax.jit(
            shard_map(
                _body,
                mesh=self.mesh,
                in_specs=(PartitionSpec("core"),) * nin,
                out_specs=(PartitionSpec("core"),) * len(out_names),
                check_rep=False,
            ),
            keep_unused=True,
        )
        self.in_names = in_names
        self.out_names = out_names
        self.out_avals = out_avals
        self.jax = jax

    def stage(self, in_maps):
        """Concatenate per-core inputs and put them on the device mesh."""
        import numpy as np
        from jax.sharding import NamedSharding, PartitionSpec

        sh = NamedSharding(self.mesh, PartitionSpec("core"))
        args = []
        for name in self.in_names:
            c = np.concatenate([m[name] for m in in_maps], axis=0)
            args.append(self.jax.device_put(c, sh))
        for av in self.out_avals:
            z = np.zeros((N_CORES * av.shape[0], *av.shape[1:]), av.dtype)
            args.append(self.jax.device_put(z, sh))
        return args

    def run(self, args):
        outs = self.fn(*args)
        self.jax.block_until_ready(outs)
        return outs

    def time(self, args, iters=8):
        import time

        self.run(args)  # warm
        t0 = time.perf_counter()
        outs = None
        for _ in range(iters):
            outs = self.fn(*args)
        self.jax.block_until_ready(outs)
        t_pipe = (time.perf_counter() - t0) / iters
        per_call = []
        for _ in range(iters):
            t0 = time.perf_counter()
            self.jax.block_until_ready(self.fn(*args))
            per_call.append(time.perf_counter() - t0)
        return t_pipe, min(per_call)


def _get_runner():
    if "runner" not in _CACHE:
        _CACHE["runner"] = _Runner(_build())
    return _CACHE["runner"]


def _in_maps(x, W1, b1, W2, b2):
    x = np.asarray(x, dtype=np.float32)
    W1 = np.asarray(W1, dtype=np.float32)
    W2 = np.asarray(W2, dtype=np.float32)
    maps = []
    for i in range(N_CORES):
        maps.append(
            {
                "x": np.ascontiguousarray(x[:, i * DC : (i + 1) * DC]),
                "W1": np.ascontiguousarray(W1[i * NBC : (i + 1) * NBC]),
                "W2": np.ascontiguousarray(W2[i * NBC : (i + 1) * NBC]),
            }
        )
    return maps


def _kernel_cpu(x, W1, b1, W2, b2):
    """Reference math on the jax CPU backend (safety fallback)."""
    import jax
    import jax.numpy as jnp

    with jax.default_device(jax.devices("cpu")[0]):
        h = jnp.asarray(x).reshape(BS, NB, BD).transpose(1, 0, 2)
        h = jnp.einsum("nbi,nio->nbo", h, jnp.asarray(W1)) + jnp.asarray(b1)
        h = jax.nn.gelu(h, approximate=False)
        h = jnp.einsum("nbi,nio->nbo", h, jnp.asarray(W2)) + jnp.asarray(b2)
        return np.asarray(h.transpose(1, 0, 2).reshape(BS, D), dtype=np.float32)


def kernel(x, W1, b1, W2, b2):
    try:
        if np.any(b1) or np.any(b2):
            return _kernel_cpu(x, W1, b1, W2, b2)
        r = _get_runner()
        args = r.stage(_in_maps(x, W1, b1, W2, b2))
        outs = r.run(args)
        full = np.asarray(outs[r.out_names.index("out")])
        # [8*8192, 512] -> [8192, 8, 512] -> [8192, 4096]
        full = full.reshape(N_CORES, BS, DC).transpose(1, 0, 2).reshape(BS, D)
        return np.ascontiguousarray(full)
    except Exception:
        import traceback

        traceback.print_exc()
        return _kernel_cpu(x, W1, b1, W2, b2)
